# revision 2
# baseline (speedup 1.0000x reference)
"""Trainium2 Bass kernel for nn_Attn_34754875359623.

Computation (B=32, S=4096, H=256):
    scores[b,s] = u[b,s,:] @ w_eff + const(b)        (softmax is shift-invariant,
    attn        = softmax(scores, axis=s)             so const(b) is dropped)
    d[b,s]      = ||u[b,s,:] - v[b,:]||_2
    w_d[b]      = sum_s d[b,s] * attn[b,s]
    returns (w_d, attn)

where w_eff = W_attn[:, :H].T @ v_param (host precompute, tiny).

Strategy (data-parallel over batch, 4 samples per core on 8 cores):
  - SWDGE DMA loads u and casts fp32->bf16 inline (halves SBUF write traffic).
  - DMA xbar transpose (bf16, SBUF->SBUF) produces u^T chunks [h, s] so the
    TensorEngine can contract over h: one matmul per (s-tile, h-chunk) with
    u^T as the stationary operand and [w_eff | v_b] as the 2-column moving
    operand -> PSUM [128 s, 2] accumulated over h-chunks.
  - ||u_s||^2 via DVE scalar_tensor_tensor accumulate on the natural-layout
    bf16 tile (sum over free dim h).
  - Small softmax / weighted-sum epilogue per sample; cross-partition sums
    done with tiny ones-vector matmuls.
"""

import numpy as np
import ml_dtypes

B, S, H = 32, 4096, 256
NCORES = 8
BL = B // NCORES  # samples per core
T = 32            # s-tiles per sample; s = j*T_idx mapping: s = j*32 + t
HC = 2            # h chunks of 128

_CACHE = {}


def _build_nc(use_dma_cast=True):
    from contextlib import ExitStack

    import concourse.bass as bass
    import concourse.bacc as bacc
    import concourse.tile as tile
    from concourse import mybir

    f32 = mybir.dt.float32
    bf16 = mybir.dt.bfloat16
    AF = mybir.ActivationFunctionType
    ALU = mybir.AluOpType

    nc = bacc.Bacc("TRN2", target_bir_lowering=False, debug=False)

    u_d = nc.dram_tensor("u", [BL, S, H], f32, kind="ExternalInput")
    # rhs[h, b, c, 0] = w_eff[c*128+h], rhs[h, b, c, 1] = v[b, c*128+h]
    rhs_d = nc.dram_tensor("rhs", [128, BL, HC, 2], bf16, kind="ExternalInput")
    vsq_d = nc.dram_tensor("vsq", [1, BL], f32, kind="ExternalInput")
    attn_d = nc.dram_tensor("attn", [BL, S], f32, kind="ExternalOutput")
    wd_d = nc.dram_tensor("wd", [1, BL], f32, kind="ExternalOutput")

    with tile.TileContext(nc) as tc, ExitStack() as ctx:
        singles = ctx.enter_context(tc.tile_pool(name="singles", bufs=1))
        nat_pool = ctx.enter_context(tc.tile_pool(name="nat", bufs=2))
        ut_pool = ctx.enter_context(tc.tile_pool(name="ut", bufs=2))
        small = ctx.enter_context(tc.tile_pool(name="small", bufs=2))
        scratch_pool = ctx.enter_context(tc.tile_pool(name="scr", bufs=2))
        psum_pool = ctx.enter_context(tc.tile_pool(name="ps", bufs=2, space="PSUM"))
        tail_psum = ctx.enter_context(tc.tile_pool(name="tps", bufs=1, space="PSUM"))

        # --- constants / params ---
        rhs_sb = singles.tile([128, BL, HC, 2], bf16)
        nc.gpsimd.dma_start(out=rhs_sb[:], in_=rhs_d.ap())
        vsqb = singles.tile([128, BL], f32)
        vsq_bcast = bass.AP(
            tensor=vsq_d.ap().tensor, offset=0, ap=[[0, 128], [1, BL]]
        )
        nc.gpsimd.dma_start(out=vsqb[:], in_=vsq_bcast)
        ones_col = singles.tile([128, 1], f32)
        nc.vector.memset(ones_col[:], 1.0)
        ones_row = singles.tile([1, 128], f32)
        nc.vector.memset(ones_row[:], 1.0)

        # --- per-core batched epilogue tiles ---
        e_all = singles.tile([128, BL, T], f32)     # exp(scores)
        d2_all = singles.tile([128, BL, T], f32)    # usq - 2uv (pre-bias)
        d_all = singles.tile([128, BL, T], f32)     # distances
        attn_sb = singles.tile([128, BL, T], f32)   # normalized attn out
        zw = singles.tile([128, 2, BL], f32)        # per-partition Z and W partials

        for b in range(BL):
            # 1. load + cast u[b] -> bf16 natural layout [j, t, h], s = j*32+t
            nat = nat_pool.tile([128, T, H], bf16 if use_dma_cast else f32)
            u_b = u_d.ap()[b].rearrange("(j t) h -> j t h", t=T)
            nc.gpsimd.dma_start(out=nat[:], in_=u_b)
            if not use_dma_cast:
                nat16 = nat_pool.tile([128, T, H], bf16, tag="nat16")
                nc.vector.tensor_copy(nat16[:], nat[:])
                nat = nat16

            # 2. xbar transposes: [128 j, 128 h] -> [128 h, 128 j] per (t, c)
            ut0 = ut_pool.tile([128, T, 128], bf16, tag="ut0")
            ut1 = ut_pool.tile([128, T, 128], bf16, tag="ut1")
            uts = (ut0, ut1)
            for t in range(T):
                for c in range(HC):
                    nc.sync.dma_start(
                        out=uts[c][:, t, :],
                        in_=nat[:, t, c * 128:(c + 1) * 128],
                        transpose=True,
                    )

            # 3. usq[j, t] = sum_h u^2 on natural tile (DVE fused mul+accum)
            usq_sb = small.tile([128, T], f32, tag="usq")
            for t in range(T):
                scr = scratch_pool.tile([128, H], bf16, tag="sqscr")
                nc.vector.scalar_tensor_tensor(
                    out=scr[:],
                    in0=nat[:, t, :],
                    scalar=1.0,
                    in1=nat[:, t, :],
                    op0=ALU.mult,
                    op1=ALU.mult,
                    accum_out=usq_sb[:, t:t + 1],
                )

            # 4. scores & uv: psum[j, t, 0:2] = sum_h uT[h, j] * rhs[h, {w,v}]
            ps = psum_pool.tile([128, T, 2], f32, tag="ps")
            for t in range(T):
                for c in range(HC):
                    nc.tensor.matmul(
                        ps[:, t, :],
                        lhsT=uts[c][:, t, :],
                        rhs=rhs_sb[:, b, c, :],
                        start=(c == 0),
                        stop=(c == HC - 1),
                    )

            # 5. per-sample epilogue
            sc = small.tile([128, T, 2], f32, tag="sc")
            nc.vector.tensor_copy(sc[:], ps[:])
            # e = exp(scores)  (scores are O(+-3): no max-subtraction needed)
            nc.scalar.activation(e_all[:, b, :], sc[:, :, 0], AF.Exp)
            # d2 = usq - 2*uv   (+ vsq added as sqrt bias)
            nc.vector.scalar_tensor_tensor(
                out=d2_all[:, b, :],
                in0=sc[:, :, 1],
                scalar=-2.0,
                in1=usq_sb[:],
                op0=ALU.mult,
                op1=ALU.add,
            )
            nc.scalar.activation(
                d_all[:, b, :], d2_all[:, b, :], AF.Sqrt, bias=vsqb[:, b:b + 1]
            )
            # p = d * e ; Z = sum_s e ; W = sum_s d*e   (partials per partition)
            p_b = small.tile([128, T], f32, tag="pb")
            nc.vector.tensor_mul(p_b[:], d_all[:, b, :], e_all[:, b, :])
            nc.vector.tensor_reduce(
                zw[:, 0, b:b + 1], e_all[:, b, :], axis=mybir.AxisListType.X,
                op=ALU.add,
            )
            nc.vector.tensor_reduce(
                zw[:, 1, b:b + 1], p_b[:], axis=mybir.AxisListType.X, op=ALU.add,
            )

        # --- tail: cross-partition sums via ones-matmuls, normalize, store ---
        zps = tail_psum.tile([1, 2 * BL], f32, tag="zps")
        nc.tensor.matmul(
            zps[:], lhsT=ones_col[:], rhs=zw[:].rearrange("p q b -> p (q b)")
        )
        zs = small.tile([1, 2 * BL], f32, tag="zs")
        nc.vector.tensor_copy(zs[:], zps[:])
        rz = small.tile([1, BL], f32, tag="rz")
        nc.vector.reciprocal(rz[:], zs[:, 0:BL])
        wd_sb = small.tile([1, BL], f32, tag="wd")
        nc.vector.tensor_mul(wd_sb[:], zs[:, BL:2 * BL], rz[:])
        nc.scalar.dma_start(out=wd_d.ap(), in_=wd_sb[:])

        # broadcast 1/Z to all partitions: [128, BL] = ones_row.T @ rz
        rzb_ps = tail_psum.tile([128, BL], f32, tag="rzb")
        nc.tensor.matmul(rzb_ps[:], lhsT=ones_row[:], rhs=rz[:])
        rzb = small.tile([128, BL], f32, tag="rzbs")
        nc.vector.tensor_copy(rzb[:], rzb_ps[:])
        for b in range(BL):
            nc.vector.tensor_scalar_mul(
                out=attn_sb[:, b, :], in0=e_all[:, b, :], scalar1=rzb[:, b:b + 1]
            )
        attn_out = attn_d.ap().rearrange("b (j t) -> j b t", t=T)
        nc.scalar.dma_start(out=attn_out, in_=attn_sb[:])

    nc.compile()
    return nc


def _get_nc():
    if "nc" not in _CACHE:
        _CACHE["nc"] = _build_nc()
    return _CACHE["nc"]


def _make_in_maps(u, v, W_attn, b_attn, v_param):
    bf16 = ml_dtypes.bfloat16
    u = np.ascontiguousarray(np.asarray(u, dtype=np.float32))
    v = np.asarray(v, dtype=np.float32)
    W_attn = np.asarray(W_attn, dtype=np.float32)
    v_param = np.asarray(v_param, dtype=np.float32)

    # w_eff[h] = sum_k W_attn[k, h] * v_param[k]  (the Wu = W_attn[:, :H] part)
    w_eff = (W_attn[:, :H].astype(np.float64) * v_param[:, None].astype(np.float64)).sum(axis=0)
    w_eff16 = w_eff.astype(np.float32).astype(bf16)
    v16 = v.astype(bf16)
    vsq = (v.astype(np.float64) ** 2).sum(axis=1).astype(np.float32)  # [B]

    in_maps = []
    for core in range(NCORES):
        b0 = core * BL
        rhs = np.empty((128, BL, HC, 2), dtype=bf16)
        for c in range(HC):
            rhs[:, :, c, 0] = w_eff16[c * 128:(c + 1) * 128][:, None]
            rhs[:, :, c, 1] = v16[b0:b0 + BL, c * 128:(c + 1) * 128].T
        in_maps.append({
            "u": u[b0:b0 + BL],
            "rhs": rhs,
            "vsq": vsq[b0:b0 + BL].reshape(1, BL),
        })
    return in_maps


def _run(in_maps, trace=False, **kwargs):
    from concourse import bass_utils
    nc = _get_nc()
    return bass_utils.run_bass_kernel_spmd(
        nc, in_maps, core_ids=list(range(NCORES)), trace=trace, **kwargs
    )


def kernel(u, v, W_attn, b_attn, v_param):
    res = _run(_make_in_maps(u, v, W_attn, b_attn, v_param))
    attn = np.concatenate([r["attn"] for r in res.results], axis=0)
    w_d = np.concatenate([r["wd"][0] for r in res.results], axis=0)
    return (w_d.astype(np.float32), attn.astype(np.float32))


# revision 6
# speedup vs baseline: 2.6147x; 2.6147x over previous
"""Trainium2 Bass kernel for nn_Attn_34754875359623.

Computation (B=32, S=4096, H=256):
    scores[b,s] = u[b,s,:] @ w_eff + const(b)        (softmax is shift-invariant,
    attn        = softmax(scores, axis=s)             so const(b) is dropped)
    d[b,s]      = ||u[b,s,:] - v[b,:]||_2  = sqrt(usq - 2*u.v + vsq)
    w_d[b]      = sum_s d[b,s] * attn[b,s]
    returns (w_d, attn)

where w_eff = W_attn[:, :H].T @ v_param (host precompute, tiny).

Strategy (data-parallel over batch, 4 samples per core on 8 cores):
  - SWDGE DMA casts u fp32 -> bf16 into a DRAM scratch, chunk-major.
  - 8 big DMA xbar transposes (DRAM [4096,128] -> SBUF [128,4096]) put h on
    partitions. (Small SBUF-sourced transposes serialize at ~1.2us each on the
    issuing engine - measured - so the DRAM round trip is worth it.)
  - TensorE contracts over h with the tiny param matrix as the stationary
    operand and u^T streaming 512 columns per matmul: psum rows {0,1} get
    {scores, u.v}; row 32 gets usq from a ones-vector matmul against u^T**2.
  - Results ([3, S] per sample) bounce through DRAM to be re-laid-out as
    [128 partitions, ...] for the softmax/weighted-sum epilogue.
"""

import numpy as np
import ml_dtypes

B, S, H = 32, 4096, 256
NCORES = 8
BL = B // NCORES  # samples per core
T = 32            # s = j*32 + t   (j = partition, t = free) in the epilogue
HC = 2            # h chunks of 128
NBLK = 2          # 512-column matmul blocks per quarter
QS = 1024         # psum quarter size (columns)

_CACHE = {}


def _build_nc():
    from contextlib import ExitStack

    import concourse.bass as bass
    import concourse.bacc as bacc
    import concourse.tile as tile
    from concourse import mybir

    f32 = mybir.dt.float32
    bf16 = mybir.dt.bfloat16
    AF = mybir.ActivationFunctionType
    ALU = mybir.AluOpType

    nc = bacc.Bacc("TRN2", target_bir_lowering=False, debug=False)

    u_d = nc.dram_tensor("u", [BL, S, H], f32, kind="ExternalInput")
    # params[h, b, c, 0] = w_eff[c*128+h], params[h, b, c, 1] = v[b, c*128+h]
    par_d = nc.dram_tensor("par", [128, BL, HC, 2], bf16, kind="ExternalInput")
    vsq_d = nc.dram_tensor("vsq", [1, BL], f32, kind="ExternalInput")
    attn_d = nc.dram_tensor("attn", [BL, S], f32, kind="ExternalOutput")
    wd_d = nc.dram_tensor("wd", [1, BL], f32, kind="ExternalOutput")

    with tile.TileContext(nc) as tc, ExitStack() as ctx:
        singles = ctx.enter_context(tc.tile_pool(name="singles", bufs=1))
        ut_pool = ctx.enter_context(tc.tile_pool(name="ut", bufs=2))
        stage_pool = ctx.enter_context(tc.tile_pool(name="stage", bufs=2))
        small = ctx.enter_context(tc.tile_pool(name="small", bufs=2))
        psum_pool = ctx.enter_context(tc.tile_pool(name="ps", bufs=3, space="PSUM"))
        tail_psum = ctx.enter_context(tc.tile_pool(name="tps", bufs=1, space="PSUM"))
        dram_pool = ctx.enter_context(tc.tile_pool(name="dram", bufs=1, space="DRAM"))

        # --- constants / params ---
        par_sb = singles.tile([128, BL, HC, 2], bf16)
        nc.gpsimd.dma_start(out=par_sb[:], in_=par_d.ap())
        vsqb = singles.tile([128, BL], f32)
        vsq_bcast = bass.AP(
            tensor=vsq_d.ap().tensor, offset=0, ap=[[0, 128], [1, BL]]
        )
        nc.gpsimd.dma_start(out=vsqb[:], in_=vsq_bcast)
        ones_bf = singles.tile([128, 1], bf16)
        nc.vector.memset(ones_bf[:], 1.0)
        ones_col = singles.tile([128, 1], f32)
        nc.vector.memset(ones_col[:], 1.0)
        ones_row = singles.tile([1, 128], f32)
        nc.vector.memset(ones_row[:], 1.0)

        # --- DRAM scratch ---
        u16 = dram_pool.tile([BL, HC, S, 128], bf16)   # chunk-major bf16 copy of u
        scb = dram_pool.tile([BL, 3, S], f32)          # {scores, uv, usq} bounce

        hwdge = [nc.sync, nc.scalar]

        for b in range(BL):
            # 1. cast u[b] fp32 -> bf16 DRAM scratch, per h-chunk (SWDGE)
            u_b = u_d.ap()[b].rearrange("s (c k) -> c s k", c=HC)
            for c in range(HC):
                nc.gpsimd.dma_start(out=u16[b, c], in_=u_b[c])

            # 2. big xbar transposes: [4096 s, 128 h] -> [128 h, 4096 s]
            ut = [ut_pool.tile([128, S], bf16, tag=f"ut{c}", name=f"ut{c}") for c in range(HC)]
            for c in range(HC):
                hwdge[c].dma_start_transpose(out=ut[c][:], in_=u16[b, c])

            # 3. squares for usq (DVE, one big op per chunk)
            u2 = [ut_pool.tile([128, S], bf16, tag=f"u2{c}", name=f"u2{c}") for c in range(HC)]
            for c in range(HC):
                nc.vector.tensor_mul(u2[c][:], ut[c][:], ut[c][:])

            # 4/5. matmuls + eviction per quarter of s
            stage = stage_pool.tile([33, S], f32, tag="stage")
            for q in range(S // QS):
                ps = psum_pool.tile([33, QS], f32, tag="ps")
                for blk in range(NBLK):
                    sl = slice(q * QS + blk * 512, q * QS + (blk + 1) * 512)
                    po = slice(blk * 512, (blk + 1) * 512)
                    for c in range(HC):
                        nc.tensor.matmul(
                            ps[0:2, po],
                            lhsT=par_sb[:, b, c, :],
                            rhs=ut[c][:, sl],
                            start=(c == 0),
                            stop=(c == HC - 1),
                        )
                    for c in range(HC):
                        nc.tensor.matmul(
                            ps[32:33, po],
                            lhsT=ones_bf[:],
                            rhs=u2[c][:, sl],
                            start=(c == 0),
                            stop=(c == HC - 1),
                        )
                nc.scalar.copy(stage[:, q * QS:(q + 1) * QS], ps[:])

            # 6. store {scores, uv} and usq rows to the DRAM bounce buffer
            nc.gpsimd.dma_start(out=scb[b, 0:2], in_=stage[0:2, :])
            nc.gpsimd.dma_start(out=scb[b, 2:3], in_=stage[32:33, :])

        # --- tail: re-layout to [128, ...], softmax + weighted sum ---
        relay = singles.tile([128, BL, 3, T], f32)
        relay_in = bass.AP(
            tensor=scb[:].tensor,
            offset=scb[:].offset,
            ap=[[T, 128], [3 * S, BL], [S, 3], [1, T]],
        )
        nc.scalar.dma_start(out=relay[:], in_=relay_in)

        e_all = singles.tile([128, BL, T], f32)
        nc.scalar.activation(e_all[:], relay[:, :, 0, :], AF.Exp)
        d2 = singles.tile([128, BL, T], f32)
        nc.vector.scalar_tensor_tensor(
            out=d2[:], in0=relay[:, :, 1, :], scalar=-2.0, in1=relay[:, :, 2, :],
            op0=ALU.mult, op1=ALU.add,
        )
        vsq_b = bass.AP(
            tensor=vsqb[:].tensor, offset=vsqb[:].offset,
            ap=[[vsqb[:].ap[0][0], 128], [1, BL], [0, T]],
        )
        nc.vector.tensor_add(d2[:], d2[:], vsq_b)
        d_all = singles.tile([128, BL, T], f32)
        nc.scalar.activation(d_all[:], d2[:], AF.Sqrt)
        p_all = singles.tile([128, BL, T], f32)
        nc.vector.tensor_mul(p_all[:], d_all[:], e_all[:])
        zw = singles.tile([128, 2, BL], f32)
        nc.vector.tensor_reduce(
            zw[:, 0], e_all[:], axis=mybir.AxisListType.X, op=ALU.add
        )
        nc.vector.tensor_reduce(
            zw[:, 1], p_all[:], axis=mybir.AxisListType.X, op=ALU.add
        )
        zps = tail_psum.tile([1, 2 * BL], f32, tag="zps")
        nc.tensor.matmul(
            zps[:], lhsT=ones_col[:], rhs=zw[:].rearrange("p q b -> p (q b)")
        )
        zs = small.tile([1, 2 * BL], f32, tag="zs")
        nc.vector.tensor_copy(zs[:], zps[:])
        rz = small.tile([1, BL], f32, tag="rz")
        nc.vector.reciprocal(rz[:], zs[:, 0:BL])
        wd_sb = small.tile([1, BL], f32, tag="wd")
        nc.vector.tensor_mul(wd_sb[:], zs[:, BL:2 * BL], rz[:])
        nc.scalar.dma_start(out=wd_d.ap(), in_=wd_sb[:])

        # broadcast 1/Z to all partitions: [128, BL] = ones_row.T @ rz
        rzb_ps = tail_psum.tile([128, BL], f32, tag="rzb")
        nc.tensor.matmul(rzb_ps[:], lhsT=ones_row[:], rhs=rz[:])
        rzb = small.tile([128, BL], f32, tag="rzbs")
        nc.vector.tensor_copy(rzb[:], rzb_ps[:])
        attn_sb = singles.tile([128, BL, T], f32)
        for b in range(BL):
            nc.vector.tensor_scalar_mul(
                out=attn_sb[:, b, :], in0=e_all[:, b, :], scalar1=rzb[:, b:b + 1]
            )
        attn_out = attn_d.ap().rearrange("b (j t) -> j b t", t=T)
        nc.scalar.dma_start(out=attn_out, in_=attn_sb[:])

    nc.compile()
    return nc


def _get_nc():
    if "nc" not in _CACHE:
        _CACHE["nc"] = _build_nc()
    return _CACHE["nc"]


def _make_in_maps(u, v, W_attn, b_attn, v_param):
    bf16 = ml_dtypes.bfloat16
    u = np.ascontiguousarray(np.asarray(u, dtype=np.float32))
    v = np.asarray(v, dtype=np.float32)
    W_attn = np.asarray(W_attn, dtype=np.float32)
    v_param = np.asarray(v_param, dtype=np.float32)

    # w_eff[h] = sum_k W_attn[k, h] * v_param[k]  (the Wu = W_attn[:, :H] part)
    w_eff = (W_attn[:, :H].astype(np.float64) * v_param[:, None].astype(np.float64)).sum(axis=0)
    w_eff16 = w_eff.astype(np.float32).astype(bf16)
    v16 = v.astype(bf16)
    vsq = (v.astype(np.float64) ** 2).sum(axis=1).astype(np.float32)  # [B]

    in_maps = []
    for core in range(NCORES):
        b0 = core * BL
        par = np.empty((128, BL, HC, 2), dtype=bf16)
        for c in range(HC):
            par[:, :, c, 0] = w_eff16[c * 128:(c + 1) * 128][:, None]
            par[:, :, c, 1] = v16[b0:b0 + BL, c * 128:(c + 1) * 128].T
        in_maps.append({
            "u": u[b0:b0 + BL],
            "par": par,
            "vsq": vsq[b0:b0 + BL].reshape(1, BL),
        })
    return in_maps


def _run(in_maps, trace=False, **kwargs):
    from concourse import bass_utils
    nc = _get_nc()
    return bass_utils.run_bass_kernel_spmd(
        nc, in_maps, core_ids=list(range(NCORES)), trace=trace, **kwargs
    )


def kernel(u, v, W_attn, b_attn, v_param):
    res = _run(_make_in_maps(u, v, W_attn, b_attn, v_param))
    attn = np.concatenate([r["attn"] for r in res.results], axis=0)
    w_d = np.concatenate([r["wd"][0] for r in res.results], axis=0)
    return (w_d.astype(np.float32), attn.astype(np.float32))
